# revision 6
# baseline (speedup 1.0000x reference)
"""AscendQwen3Attention (T=2048, HIDDEN=2048, HQ=32, HK=8, D=128) on 8 TRN2 cores.

Tensor-parallel over heads: core i owns q-heads [4i..4i+3] and kv-head i
(GQA rep=4 aligns exactly), w_qkv column-sharded to [2048, 768] per core,
w_o row-sharded to [512, 2048] per core. Each core computes a full [T, HIDDEN]
partial of the output projection (bf16); the host sums the 8 partials.

Single continuous PE stream per core, interleaving four kinds of segments so
the tensor engine never idles and stays at max p-state:
  [QKV tiles 0-3] [attn chunk0] [QKV 4-7] [op0 + attn1] [QKV 8-11]
  [op1 + attn2] [QKV 12-15] [op2 + attn3] [op3]

Per-tile QKV: 16+16 bf16 matmuls -> PSUM; ACT (copy) evicts q/k/v to bf16
SBUF; DVE computes per-head sum(x^2); ACT computes rsqrt via exp(-0.5*ln(v))
(Ln/Exp/Copy live in ONE activation table -> zero table reloads all kernel);
DVE does norm (x*istd*w) and rope, all bf16 (2-4x DVE modes); PE transposes
q/k heads to [d, t], deferred one tile so rope overlaps next tile's matmuls.

Attention per (chunk of 512 q, head): S^T[k,q] = K^T.T @ Q^T -> PSUM,
exp on ACT -> bf16 pt, 128-wide triangular mask on diagonal blocks only (DVE),
then num^T[d,q] += V.T @ pt and den[1,q] += ones.T @ pt. S matmuls issued
3 blocks ahead of AV so ACT exp latency never stalls PE. den reciprocal
(DVE) -> partition broadcast (gpsimd) -> at = num * (1/den) bf16 (DVE).

Out-proj per chunk: 16 ho-tiles x 4 head matmuls, PSUM evicted via ACT copies
smeared between matmuls, DMA'd out as bf16 [2048, 2048] partials.

PSUM budget (8 banks): ab(2: qkv A/B + transposes) + so(3: S pipeline +
outproj) + o(2: AV accum) + den(1).
"""

import os
import sys

sys.path.insert(0, "/opt/trn_rl_repo")

import numpy as np
from ml_dtypes import bfloat16

import concourse.bass as bass
import concourse.bacc as bacc
import concourse.tile as tile
import concourse.mybir as mybir
from concourse.bass_utils import run_bass_kernel_spmd

F32 = mybir.dt.float32
BF16 = mybir.dt.bfloat16
AF = mybir.ActivationFunctionType
ALU = mybir.AluOpType

T = 2048
HIDDEN = 2048
HQ, HK, D = 32, 8, 128
HALF = D // 2
MROPE = (16, 24, 24)
THETA = 1.0e6
EPS = 1e-6
N_CORES = 8
HQL = HQ // N_CORES            # 4 q heads per core
NH = HQL + 1                   # q heads + k head get rope/norm
FQKV = (HQL + 2) * D           # 768 qkv features per core
KT = HIDDEN // 128             # 16 contraction tiles
TT = T // 128                  # 16 token tiles
NQC = 4                        # q-chunks of 512
QCW = T // NQC                 # 512
SCALE = float(D) ** -0.5
CW = NH * HALF                 # 320 cos cols per t-tile

_CACHED = {}


def _build():
    nc = bacc.Bacc("TRN2", target_bir_lowering=False, debug=False,
                   num_devices=N_CORES)

    # ht packed tile-major: [128, (t_tile, kt, 128)]
    ht_d = nc.dram_tensor("ht", [128, TT * KT * 128], BF16, kind="ExternalInput")
    wqkv_d = nc.dram_tensor("wqkv", [128, KT * FQKV], BF16, kind="ExternalInput")
    wo_d = nc.dram_tensor("wo", [128, HQL * HIDDEN], BF16, kind="ExternalInput")
    cos_d = nc.dram_tensor("cos", [128, TT * CW], BF16, kind="ExternalInput")
    sin_d = nc.dram_tensor("sin", [128, TT * CW], BF16, kind="ExternalInput")
    qnw_d = nc.dram_tensor("qnw", [128, 128], BF16, kind="ExternalInput")
    knw_d = nc.dram_tensor("knw", [128, 128], BF16, kind="ExternalInput")
    mask_d = nc.dram_tensor("mask", [128, 128], BF16, kind="ExternalInput")
    ident_d = nc.dram_tensor("ident", [128, 128], BF16, kind="ExternalInput")
    out_d = nc.dram_tensor("out", [HIDDEN, T], BF16, kind="ExternalOutput")
    out_tiled = out_d.ap().rearrange("(a p) b -> a p b", p=128)

    with tile.TileContext(nc) as tc:
        with (
            tc.tile_pool(name="cst", bufs=1) as cst,
            tc.tile_pool(name="big", bufs=1) as big,
            tc.tile_pool(name="wrk", bufs=2) as wrk,
            tc.tile_pool(name="pab", bufs=2, space="PSUM") as pab,
            tc.tile_pool(name="pso", bufs=3, space="PSUM") as pso,
            tc.tile_pool(name="pacc", bufs=2, space="PSUM") as pacc,
            tc.tile_pool(name="pden", bufs=1, space="PSUM") as pden,
        ):
            # ---- persistent SBUF images -------------------------------------
            ht_sb = big.tile([128, TT * KT * 128], BF16, tag="ht")
            wqkv_sb = big.tile([128, KT * FQKV], BF16, tag="wqkv")
            wo_sb = big.tile([128, HQL * HIDDEN], BF16, tag="wo")
            cos_sb = cst.tile([128, TT * CW], BF16, tag="cos")
            sin_sb = cst.tile([128, TT * CW], BF16, tag="sin")
            qnw_sb = cst.tile([128, 128], BF16, tag="qnw")
            knw_sb = cst.tile([128, 128], BF16, tag="knw")
            mask_sb = cst.tile([128, 128], BF16, tag="mask")
            ident_sb = cst.tile([128, 128], BF16, tag="ident")
            ones_sb = cst.tile([128, 1], BF16, tag="ones")
            eps_sb = cst.tile([128, 1], F32, tag="eps")
            kt_sb = big.tile([128, T], BF16, tag="ktr")      # K^T [d, t]
            v_sb = big.tile([128, T], BF16, tag="vsb")       # V   [t, d] tiled
            qt_sb = [big.tile([128, T], BF16, tag=f"qt{h}", name=f"qt{h}")
                     for h in range(HQL)]

            # weights first (first matmul needs wqkv kt=0), then ht per tile
            for kt in range(KT):
                nc.sync.dma_start(wqkv_sb[:, kt * FQKV:(kt + 1) * FQKV],
                                  wqkv_d.ap()[:, kt * FQKV:(kt + 1) * FQKV])
            for t in range(TT):
                c0 = t * KT * 128
                nc.sync.dma_start(ht_sb[:, c0:c0 + KT * 128],
                                  ht_d.ap()[:, c0:c0 + KT * 128])
            nc.sync.dma_start(cos_sb[:], cos_d.ap())
            nc.sync.dma_start(sin_sb[:], sin_d.ap())
            nc.sync.dma_start(qnw_sb[:], qnw_d.ap())
            nc.sync.dma_start(knw_sb[:], knw_d.ap())
            nc.sync.dma_start(mask_sb[:], mask_d.ap())
            nc.sync.dma_start(ident_sb[:], ident_d.ap())
            nc.sync.dma_start(wo_sb[:], wo_d.ap())
            nc.vector.memset(ones_sb[:], 1.0)
            nc.vector.memset(eps_sb[:], EPS)

            rots = {}

            def qkv_tile(t):
                """QKV matmuls + evict + norm + rope for t-tile t (no transposes)."""
                hcol = t * KT * 128
                psA = pab.tile([128, 512], F32, tag="ab", name=f"psA_{t}")
                for kt in range(KT):
                    nc.tensor.matmul(
                        psA[:], ht_sb[:, hcol + kt * 128: hcol + kt * 128 + 128],
                        wqkv_sb[:, kt * FQKV: kt * FQKV + 512],
                        start=(kt == 0), stop=(kt == KT - 1))
                xq = wrk.tile([128, 512], BF16, tag="xq", name=f"xq_{t}")
                nc.scalar.copy(xq[:], psA[:])
                psB = pab.tile([128, 512], F32, tag="ab", name=f"psB_{t}")
                for kt in range(KT):
                    nc.tensor.matmul(
                        psB[:, 0:256],
                        ht_sb[:, hcol + kt * 128: hcol + kt * 128 + 128],
                        wqkv_sb[:, kt * FQKV + 512: kt * FQKV + 768],
                        start=(kt == 0), stop=(kt == KT - 1))
                xk = wrk.tile([128, 128], BF16, tag="xk", name=f"xk_{t}")
                nc.scalar.copy(xk[:], psB[:, 0:128])
                nc.scalar.copy(v_sb[:, t * 128:(t + 1) * 128], psB[:, 128:256])

                def head_x(h):
                    return xq[:, h * 128:(h + 1) * 128] if h < HQL else xk[:]

                # per-head sum(x^2) on DVE (bf16 ops), rsqrt via ln+exp on ACT
                sq = wrk.tile([128, 128], BF16, tag="sq", name=f"sq_{t}")
                ssq = wrk.tile([128, 8], F32, tag="ssq", name=f"ssq_{t}")
                for h in range(NH):
                    nc.vector.scalar_tensor_tensor(
                        sq[:], head_x(h), 1.0, head_x(h),
                        op0=ALU.mult, op1=ALU.mult,
                        accum_out=ssq[:, h:h + 1])
                lnv = wrk.tile([128, 8], F32, tag="lnv", name=f"lnv_{t}")
                nc.scalar.activation(lnv[:, 0:NH], ssq[:, 0:NH], AF.Ln,
                                     scale=1.0 / D, bias=eps_sb[:])
                istd = wrk.tile([128, 8], F32, tag="istd", name=f"istd_{t}")
                nc.scalar.activation(istd[:, 0:NH], lnv[:, 0:NH], AF.Exp,
                                     scale=-0.5)
                xn = wrk.tile([128, NH * 128], BF16, tag="xn", name=f"xn_{t}")
                for h in range(NH):
                    nc.vector.scalar_tensor_tensor(
                        xn[:, h * 128:(h + 1) * 128], head_x(h),
                        istd[:, h:h + 1],
                        qnw_sb[:] if h < HQL else knw_sb[:],
                        op0=ALU.mult, op1=ALU.mult)

                # batched rope over the 5 heads (strided 3D APs, all bf16)
                rot = wrk.tile([128, NH * 128], BF16, tag="rot", name=f"rot_{t}")
                xr = xn[:].rearrange("p (h d) -> p h d", h=NH)
                rr = rot[:].rearrange("p (h d) -> p h d", h=NH)
                c5 = cos_sb[:, t * CW:(t + 1) * CW].rearrange(
                    "p (h d) -> p h d", h=NH)
                s5 = sin_sb[:, t * CW:(t + 1) * CW].rearrange(
                    "p (h d) -> p h d", h=NH)
                x1 = xr[:, :, 0:HALF]
                x2 = xr[:, :, HALF:D]
                ta = wrk.tile([128, NH * HALF], BF16, tag="ta", name=f"ta_{t}")
                tb = wrk.tile([128, NH * HALF], BF16, tag="tb", name=f"tb_{t}")
                tar = ta[:].rearrange("p (h d) -> p h d", h=NH)
                tbr = tb[:].rearrange("p (h d) -> p h d", h=NH)
                nc.vector.tensor_mul(tar, x1, c5)
                nc.vector.tensor_mul(tbr, x2, s5)
                nc.vector.tensor_sub(rr[:, :, 0:HALF], tar, tbr)
                tc2 = wrk.tile([128, NH * HALF], BF16, tag="ta", name=f"tc_{t}")
                td = wrk.tile([128, NH * HALF], BF16, tag="tb", name=f"td_{t}")
                tcr = tc2[:].rearrange("p (h d) -> p h d", h=NH)
                tdr = td[:].rearrange("p (h d) -> p h d", h=NH)
                nc.vector.tensor_mul(tcr, x2, c5)
                nc.vector.tensor_mul(tdr, x1, s5)
                nc.vector.tensor_add(rr[:, :, HALF:D], tcr, tdr)
                rots[t] = rot

            def qkv_transposes(t):
                rot = rots.pop(t)
                for h in range(NH):
                    tp = pab.tile([128, 128], BF16, tag="ab", name=f"tp_{t}_{h}")
                    nc.tensor.transpose(tp[:], rot[:, h * 128:(h + 1) * 128],
                                        ident_sb[:])
                    if h < HQL:
                        nc.vector.tensor_copy(
                            qt_sb[h][:, t * 128:(t + 1) * 128], tp[:])
                    else:
                        nc.vector.tensor_copy(
                            kt_sb[:, t * 128:(t + 1) * 128], tp[:])

            at_tiles = {}

            def attn_chunk(g):
                """Causal attention for q-chunk g (512 q), heads sequential,
                S issued 3 key-blocks ahead of AV."""
                nkb = 4 * g + 4
                for h in range(HQL):
                    o_ps = pacc.tile([128, QCW], F32, tag="o", name=f"o_{g}_{h}")
                    den_ps = pden.tile([1, QCW], F32, tag="den",
                                       name=f"den_{g}_{h}")
                    pend = []

                    def issue_s(kb):
                        r = kb - 4 * g
                        q0 = 128 * r if r > 0 else 0
                        s_ps = pso.tile([128, QCW], F32, tag="so",
                                        name=f"s_{g}_{h}_{kb}")
                        nc.tensor.matmul(s_ps[:, q0:QCW],
                                         kt_sb[:, kb * 128:(kb + 1) * 128],
                                         qt_sb[h][:, g * QCW + q0:
                                                   (g + 1) * QCW],
                                         start=True, stop=True)
                        pt = wrk.tile([128, QCW], BF16, tag="pt", bufs=6,
                                      name=f"pt_{g}_{h}_{kb}")
                        nc.scalar.activation(pt[:, q0:QCW], s_ps[:, q0:QCW],
                                             AF.Exp, scale=SCALE)
                        if r >= 0:
                            nc.vector.tensor_mul(
                                pt[:, q0:q0 + 128], pt[:, q0:q0 + 128],
                                mask_sb[:])
                        pend.append((kb, pt, q0))

                    def issue_av():
                        kb, pt, q0 = pend.pop(0)
                        nc.tensor.matmul(o_ps[:, q0:QCW],
                                         v_sb[:, kb * 128:(kb + 1) * 128],
                                         pt[:, q0:QCW], start=(kb == 0),
                                         stop=(kb == nkb - 1),
                                         skip_group_check=True)
                        nc.tensor.matmul(den_ps[0:1, q0:QCW], ones_sb[:, 0:1],
                                         pt[:, q0:QCW], start=(kb == 0),
                                         stop=(kb == nkb - 1),
                                         skip_group_check=True)

                    for kb in range(nkb):
                        issue_s(kb)
                        if kb >= 3:
                            issue_av()
                    while pend:
                        issue_av()

                    den_r = wrk.tile([1, QCW], F32, tag="denr",
                                     name=f"denr_{g}_{h}")
                    nc.vector.reciprocal_approx_fast(den_r[0:1, :],
                                                     den_ps[0:1, :])
                    den_b = wrk.tile([128, QCW], F32, tag="denb",
                                     name=f"denb_{g}_{h}")
                    nc.gpsimd.partition_broadcast(den_b[:], den_r[0:1, :])
                    at = wrk.tile([128, QCW], BF16, tag="at", bufs=8,
                                  name=f"at_{g}_{h}")
                    # deferred: at-mul for head h-1 issued during head h's
                    # stream so the gpsimd broadcast latency is hidden
                    if h > 0:
                        _flush_at(g, h - 1)
                    at_tiles[(g, h)] = (at, o_ps, den_b)
                _flush_at(g, HQL - 1)

            def _flush_at(g, h):
                at, o_ps, den_b = at_tiles[(g, h)]
                if o_ps is not None:
                    nc.vector.tensor_mul(at[:], o_ps[:], den_b[:])
                    at_tiles[(g, h)] = (at, None, None)

            def outproj(g):
                prev_osb = None
                for ho in range(TT):
                    op = pso.tile([128, QCW], F32, tag="so",
                                  name=f"op_{g}_{ho}")
                    for f in range(HQL):
                        nc.tensor.matmul(
                            op[:],
                            wo_sb[:, f * HIDDEN + ho * 128:
                                  f * HIDDEN + ho * 128 + 128],
                            at_tiles[(g, f)][0][:],
                            start=(f == 0), stop=(f == HQL - 1))
                    if prev_osb is not None:
                        _evict_osb(g, ho - 1, prev_osb)
                    prev_osb = op
                _evict_osb(g, TT - 1, prev_osb)

            def _evict_osb(g, ho, op):
                osb = wrk.tile([128, QCW], BF16, tag="osb", bufs=4,
                               name=f"osb_{g}_{ho}")
                nc.scalar.copy(osb[:], op[:])
                nc.sync.dma_start(
                    out_tiled[ho][:, g * QCW:(g + 1) * QCW], osb[:])

            # ---- main schedule ---------------------------------------------
            for g in range(NQC):
                for t in range(4 * g, 4 * g + 4):
                    qkv_tile(t)
                    if t - 1 in rots:
                        qkv_transposes(t - 1)
                if g > 0:
                    outproj(g - 1)
                qkv_transposes(4 * g + 3)
                attn_chunk(g)
            outproj(NQC - 1)

    nc.compile()
    return nc


def _pack_rows(a):
    """[N*128, M] -> [128, N*M] SBUF image (partition-major k-tiles)."""
    n = a.shape[0] // 128
    return np.ascontiguousarray(
        a.reshape(n, 128, a.shape[1]).transpose(1, 0, 2).reshape(128, -1))


def _pack_ht(hsT):
    """hidden^T [2048, 2048] -> [128, (t_tile, kt, 128)] bf16."""
    # hsT[kt*128+p, tile*128+c] -> img[p, ((tile*KT)+kt)*128 + c]
    a = hsT.reshape(KT, 128, TT, 128)          # [kt, p, tile, c]
    a = a.transpose(1, 2, 0, 3)                # [p, tile, kt, c]
    return np.ascontiguousarray(a.reshape(128, -1))


def _cos_sin(positions):
    j = np.arange(HALF, dtype=np.float32)
    inv_freq = (np.float32(THETA) ** (-j / np.float32(HALF))).astype(np.float32)
    pos = positions.astype(np.float32)
    freqs3 = pos[:, :, None] * inv_freq[None, None, :]      # [3, T, HALF] f32
    sel = np.zeros(HALF, dtype=np.int64)
    sel[MROPE[0]:MROPE[0] + MROPE[1]] = 1
    sel[MROPE[0] + MROPE[1]:] = 2
    freqs = freqs3[sel, :, np.arange(HALF)].T               # [T, HALF]
    freqs = np.ascontiguousarray(freqs.astype(np.float32))
    return np.cos(freqs).astype(np.float32), np.sin(freqs).astype(np.float32)


def _prep_inputs(hidden_states, positions, w_qkv, w_o, q_norm_w, k_norm_w):
    ht = _pack_ht(np.ascontiguousarray(hidden_states.T).astype(bfloat16))
    cos, sin = _cos_sin(positions)
    cos_p = _pack_rows(np.tile(cos, (1, NH)).astype(bfloat16))
    sin_p = _pack_rows(np.tile(sin, (1, NH)).astype(bfloat16))
    qnw = np.tile(np.asarray(q_norm_w, np.float32)[None, :], (128, 1)
                  ).astype(bfloat16)
    knw = np.tile(np.asarray(k_norm_w, np.float32)[None, :], (128, 1)
                  ).astype(bfloat16)
    # diag mask: keys on partitions p, q offset j in the 128-wide diag block:
    # visible iff j >= p
    mask = (np.arange(128)[None, :] >= np.arange(128)[:, None]).astype(bfloat16)
    ident = np.eye(128, dtype=bfloat16)

    in_maps = []
    for i in range(N_CORES):
        q0 = HQL * i * D
        wq = w_qkv[:, q0: q0 + HQL * D]
        wk = w_qkv[:, HQ * D + i * D: HQ * D + (i + 1) * D]
        wv = w_qkv[:, (HQ + HK) * D + i * D: (HQ + HK) * D + (i + 1) * D]
        wqkv_i = np.concatenate([wq, wk, wv], axis=1).astype(bfloat16)
        wo_i = w_o[HQL * i * D: HQL * (i + 1) * D, :].astype(bfloat16)
        in_maps.append({
            "ht": ht,
            "wqkv": _pack_rows(wqkv_i),
            "wo": _pack_rows(wo_i),
            "cos": cos_p,
            "sin": sin_p,
            "qnw": qnw,
            "knw": knw,
            "mask": mask,
            "ident": ident,
        })
    return in_maps


LAST_RESULTS = None


def kernel(**inputs):
    global LAST_RESULTS
    if "nc" not in _CACHED:
        _CACHED["nc"] = _build()
    nc = _CACHED["nc"]
    in_maps = _prep_inputs(**{k: np.asarray(v) for k, v in inputs.items()})
    trace = bool(os.environ.get("BASS_TRACE"))
    res = run_bass_kernel_spmd(nc, in_maps, core_ids=list(range(N_CORES)),
                               trace=trace)
    LAST_RESULTS = res
    acc = np.zeros((HIDDEN, T), dtype=np.float32)
    for i in range(N_CORES):
        acc += res.results[i]["out"].astype(np.float32)
    return np.ascontiguousarray(acc.T)
